# revision 43
# baseline (speedup 1.0000x reference)
"""CAM (channel attention) module kernel for Trainium2, data-parallel over batch.

Computes, per sample:
    v = x.reshape(C, N)                  # N = H*W
    energy = v @ v.T                     # [C, C]
    att = softmax(rowmax(energy) - energy, axis=-1)
    out = gamma * (att @ v) + x

Distribution: batch B=32 split over 8 NeuronCores (4 samples/core), gamma
replicated.  Per core everything is computed on-chip:
  - v loaded once to SBUF (doubles as x for the residual add)
  - v^T built with PE transpose-mode matmuls (needed for the energy matmul,
    whose contraction is over the spatial axis)
  - energy is SYMMETRIC (E = V V^T), so only the upper-triangle block rows
    are computed (free widths 512/384/256/256 per 128-row block), with
    float32r (FP22) matmuls accumulated in PSUM
  - softmax via a GLOBAL-bias exponent: with any per-sample constant G,
    Q = exp(G - E) is symmetric, so Q doubles as the *transposed*
    unnormalized attention (the per-row max/min bias of the reference
    softmax cancels exactly against the row normalizer).  Row sums of Q
    (ACT accum_out) give Z'.  Then out = (Q @ v) * (gamma / Z') + x.
    This removes the attention transpose + per-row min reductions of the
    direct formulation entirely.
  - G is derived on-chip from block-0's row minima: G = (min+max)/2 - 10,
    computed with one DVE row-reduce + two gpsimd partition_all_reduce ops;
    sample i uses sample i-1's G (the safety window is far wider than the
    cross-sample drift) so the exps never wait on the reduction chain
    (numerics: exponents stay within +-83, Q in [1e-36, 8e35], validated)
  - the 5 lower-triangle Q blocks are mirrored from the upper ones with PE
    transposes; the DVE copy back to SBUF also row-sums them (accum_out) to
    complete Z'
  - epilogue fuses (psum * (gamma/Z')) + x in one DVE pass
"""

import sys

sys.path.insert(0, "/opt/trn_rl_repo")

from contextlib import ExitStack

import numpy as np

import concourse.bacc as bacc
import concourse.bass as bass
import concourse.bass_isa as bass_isa
import concourse.mybir as mybir
import concourse.tile as tile
from concourse import masks
from concourse.bass_utils import run_bass_kernel_spmd

B, C, H, W = 32, 512, 48, 48
N = H * W  # 2304
NCORES = 8
SPC = B // NCORES  # samples per core
P = 128
CB = C // P  # 4 channel blocks
KB = N // P  # 18 spatial chunks of 128
NCH = [512, 512, 512, 512, 256]  # free-dim chunking of N for the 2nd matmul
G_SHIFT = 10.0  # centers exp(G - E) in fp32 range (see module docstring)

# stored (upper-triangle) energy row-block geometry: row ib holds cols
# [CLO[ib]*P, 512) of the attention matrix
CLO = [0, 1, 2, 2]  # first stored 128-col block per row (row 3 keeps 2 blocks
#                     so the matmul free dim stays >= 256, the fp32r fast path)
MIRROR = [(1, 0), (2, 0), (2, 1), (3, 0), (3, 1)]  # (ib, jb): fill from (jb, ib)^T

FP32 = mybir.dt.float32
FP32R = mybir.dt.float32r
AX = mybir.AxisListType.X
OP = mybir.AluOpType
AF = mybir.ActivationFunctionType
RED = bass_isa.ReduceOp


def _emit(tc, ctx, x, gamma, out, reps=1):
    nc = tc.nc

    const_pool = ctx.enter_context(tc.tile_pool(name="const", bufs=1))
    ident_f32 = const_pool.tile([P, P], FP32)
    masks.make_identity(nc, ident_f32[:])
    ident = const_pool.tile([P, P], FP32R)
    nc.scalar.copy(ident[:], ident_f32[:])
    gamma_sb = const_pool.tile([P, 1], FP32)

    v_pool = ctx.enter_context(tc.tile_pool(name="v", bufs=3))
    vt_pool = ctx.enter_context(tc.tile_pool(name="vt", bufs=1))
    q_pool = ctx.enter_context(tc.tile_pool(name="q", bufs=2))
    o_pool = ctx.enter_context(tc.tile_pool(name="o", bufs=3))
    vec_pool = ctx.enter_context(tc.tile_pool(name="vec", bufs=4))
    s_pool = ctx.enter_context(tc.tile_pool(name="s", bufs=2))
    # PSUM budget exactly 8 banks: 4 energy (one per row block), 2 rotating
    # transpose banks (v^T chunks and Q mirrors), 2 output banks.
    ps_e = ctx.enter_context(tc.tile_pool(name="ps_e", bufs=1, space="PSUM"))
    ps_t = ctx.enter_context(tc.tile_pool(name="ps_t", bufs=2, space="PSUM"))
    ps_o = ctx.enter_context(tc.tile_pool(name="ps_o", bufs=2, space="PSUM"))

    nsamp = reps * SPC
    v_t = {}
    vt_t = {}
    g_t = {}

    # v loads are range-major (all 4 c-blocks of a column range together) so
    # the k-th transpose chunk only needs the first few ranges.  The first
    # range is a single 128-col chunk so the first transpose starts early.
    VB = [0, 128] + [128 + 384 * r for r in range(1, 6)] + [N]
    # sample 0's transposes consume chunks just-in-time at ~0.9us each, so
    # its ranges are finer up front to match delivery to consumption
    VB0 = [0, 128, 256, 384, 640, 896, 1280, 1664, 2048, N]

    def load_v_alloc(i):
        v_t[i] = v_pool.tile([P, CB * N], FP32R, tag="v", name=f"v{i}")

    def load_v_range(i, r):
        # ONE dma per range covering all 4 c-blocks (3D access pattern):
        # each dma_start holds the issuing queue's sequencer ~0.6us, so
        # instruction count — not bytes — set the issue rate.  All on the SP
        # queue: ACT must stay DMA-free so the latency-critical exps are
        # never stuck behind a DMA's descriptor-generation hold.
        s, v = i % SPC, v_t[i]
        vb = VB0 if i == 0 else VB
        a, b = vb[r], vb[r + 1]
        dst = v.rearrange("p (cb n) -> p cb n", cb=CB)[:, :, a:b]
        src = x[s].rearrange("(cb p) n -> p cb n", p=P)[:, :, a:b]
        nc.sync.dma_start(dst, src.bitcast(FP32R))

    def a_chunk(i, k, copy_eng=None):
        # transpose one 128-wide spatial chunk of v into vt
        if k == 0:
            vt_t[i] = vt_pool.tile([P, KB * C], FP32R, tag="vt", name=f"vt{i}")
        v, vt = v_t[i], vt_t[i]
        tps = ps_t.tile([P, 512], FP32R, tag="tps")
        for cb in range(CB):
            nc.tensor.matmul(
                tps[:, cb * P : (cb + 1) * P],
                v[:, cb * N + k * P : cb * N + (k + 1) * P],
                ident[:],
                is_transpose=True,
                start=(cb == 0),
                stop=(cb == CB - 1),
            )
        if copy_eng == "dve":
            nc.vector.tensor_copy(vt[:, k * C : (k + 1) * C], tps[:])
        elif copy_eng == "act":
            nc.scalar.copy(vt[:, k * C : (k + 1) * C], tps[:])
        else:
            nc.any.tensor_copy(vt[:, k * C : (k + 1) * C], tps[:])

    def emit(i):
        s = i % SPC
        v, vt = v_t[i], vt_t.get(i)
        if i + 1 < nsamp:
            load_v_alloc(i + 1)

        # ---- upper-triangle energy: row block ib holds cols [CLO[ib]*P, 512)
        # one PSUM bank per row block so starting row ib+1's accumulation
        # group never serializes against row ib's softmax readers
        energy = [
            ps_e.tile([P, 512], FP32, tag=f"eb{ib}", name=f"eb{ib}_{i}")
            for ib in range(CB)
        ]
        q_sb = q_pool.tile([P, CB * 512], FP32R, tag="q")
        z = vec_pool.tile([P, CB], FP32, tag="z")
        s_all = s_pool.tile([P, CB], FP32, tag="s")

        def row_rgn(ib):
            return slice(ib * 512 + CLO[ib] * P, (ib + 1) * 512)

        def energy_mm(ib, k):
            lo = CLO[ib] * P
            nc.tensor.matmul(
                energy[ib][:, lo:512],
                vt_t[i][:, k * C + ib * P : k * C + (ib + 1) * P],
                vt_t[i][:, k * C + lo : k * C + C],
                start=(k == 0),
                stop=(k == KB - 1),
            )

        if i == 0:
            # prologue: v DMAs are still landing — build vt chunks just in
            # time and advance all 4 row blocks per chunk (k-outer) so the
            # PE keeps pace with the DMA stream
            for k in range(KB):
                if k == 0:
                    a_chunk(0, 0)
                if k + 1 < KB:
                    a_chunk(0, k + 1)
                for ib in range(CB):
                    energy_mm(ib, k)
        for ib in range(CB):
            lo = CLO[ib] * P
            if i > 0:
                for k in range(KB):
                    energy_mm(ib, k)
            if ib == 0:
                # ---- global bias G = (min+max of block-0 row minima)/2 - 10
                mn0 = vec_pool.tile([P, 1], FP32, tag="mn0")
                nc.vector.tensor_reduce(
                    mn0[:], energy[0][:, 0:512], axis=AX, op=OP.min
                )
                nm0 = vec_pool.tile([P, 1], FP32, tag="nm0")
                nc.vector.tensor_scalar(nm0[:], mn0[:], -1.0, None, OP.mult)
                mx_r = vec_pool.tile([P, 1], FP32, tag="mx_r")
                nc.gpsimd.partition_all_reduce(mx_r[:], mn0[:], P, RED.max)
                nmn_r = vec_pool.tile([P, 1], FP32, tag="nmn_r")
                nc.gpsimd.partition_all_reduce(nmn_r[:], nm0[:], P, RED.max)
                g_sb = vec_pool.tile([P, 1], FP32, tag="g_sb")
                nc.vector.tensor_tensor(g_sb[:], mx_r[:], nmn_r[:], op=OP.subtract)
                nc.vector.tensor_scalar(
                    g_sb[:], g_sb[:], 0.5, -G_SHIFT, OP.mult, OP.add
                )
                g_t[i] = g_sb
            # ---- Q = exp(G - E) over the stored region; accum -> partial Z'
            # G comes from the PREVIOUS sample (validated numerically: the
            # safety window is far wider than the cross-sample drift), so the
            # exps never wait on this sample's G reduction chain
            nc.scalar.activation(
                q_sb[:, row_rgn(ib)],
                energy[ib][:, lo:512],
                AF.Exp,
                bias=g_t[max(i - 1, 0)][:],
                scale=-1.0,
                accum_out=z[:, ib : ib + 1],
            )

        # a few of the next sample's transposes fill the exp tail; their
        # PSUM->SBUF copies go on ACT (after the exps in its stream) so the
        # DVE can run the mirror copies that gate the first output block
        if i + 1 < nsamp:
            for r in range(3):
                load_v_range(i + 1, r)
            a_queue = list(range(KB))
            for _ in range(3):
                a_chunk(i + 1, a_queue.pop(0), copy_eng="act")
        else:
            a_queue = []

        # ---- mirror lower-triangle Q blocks (Q is symmetric); the copy back
        # also row-sums them to complete Z'
        pps = {}
        for ib, jb in MIRROR:
            tps = ps_t.tile([P, 512], FP32R, tag="tps")
            nc.tensor.matmul(
                tps[:, 0:P],
                q_sb[:, jb * 512 + ib * P : jb * 512 + (ib + 1) * P],
                ident[:],
                is_transpose=True,
                start=True,
                stop=True,
            )
            pp = vec_pool.tile([P, 1], FP32, tag=f"pp{ib}{jb}")
            nc.vector.tensor_scalar(
                q_sb[:, ib * 512 + jb * P : ib * 512 + (jb + 1) * P],
                tps[:, 0:P],
                0.0,
                None,
                OP.add,
                OP.add,  # accum reduction op
                accum_out=pp[:],
            )
            pps.setdefault(ib, []).append(pp)

        # ---- s = gamma / Z'
        for ib in range(CB):
            zt = z[:, ib : ib + 1]
            for pp in pps.get(ib, []):
                zn = vec_pool.tile([P, 1], FP32, tag=f"zn{ib}")
                nc.vector.tensor_tensor(zn[:], zt, pp[:], op=OP.add)
                zt = zn[:]
            r = vec_pool.tile([P, 1], FP32, tag=f"r{ib}")
            nc.vector.reciprocal(r[:], zt)
            nc.vector.tensor_tensor(
                s_all[:, ib : ib + 1], r[:], gamma_sb[:], op=OP.mult
            )

        # ---- out = (Q^T-as-stored @ v) * (gamma/Z') + x
        # chunk-outer so the 4 c-blocks of one n-chunk can be stored with a
        # SINGLE combined DMA (3D access pattern): 5 output DMAs per sample
        # instead of 20 keeps the SP issue queue (~0.6us/dma) off the
        # critical path
        last = i == nsamp - 1
        n_off = 0
        for ich, nch in enumerate(NCH):
            ot = o_pool.tile([P, CB * 512], FP32, tag="ot")
            for cb in range(CB):
                it = ich * CB + cb
                if i + 1 < nsamp and it in (2, 5, 8, 11):
                    load_v_range(i + 1, 3 + (it - 2) // 3)
                if a_queue:
                    # alternate the PSUM->SBUF copies between ACT and DVE so
                    # neither queue's backlog delays the next sample's energy
                    a_chunk(i + 1, a_queue.pop(0), copy_eng="act" if it % 2 else "dve")
                if last:
                    # no next-sample fillers exist to pace the PE, so rotate
                    # the output accumulators through the freed energy banks
                    # (4-deep) instead of the 2 ps_o banks
                    po = ps_e.tile([P, 512], FP32, tag=f"eb{it % CB}")
                else:
                    po = ps_o.tile([P, 512], FP32, tag="po")
                for db in range(CB):
                    nc.tensor.matmul(
                        po[:, :nch],
                        q_sb[:, db * 512 + cb * P : db * 512 + (cb + 1) * P],
                        v[:, db * N + n_off : db * N + n_off + nch],
                        start=(db == 0),
                        stop=(db == CB - 1),
                    )
                nc.vector.scalar_tensor_tensor(
                    ot[:, cb * 512 : cb * 512 + nch],
                    po[:, :nch],
                    s_all[:, cb : cb + 1],
                    v[:, cb * N + n_off : cb * N + n_off + nch].bitcast(FP32),
                    op0=OP.mult,
                    op1=OP.add,
                )
                if last:
                    # per-block stores at the end: the kernel tail is the last
                    # epilogue->DMA chain, so don't serialize 4 epilogues
                    # into one big store (SP has issue slack by now)
                    nc.sync.dma_start(
                        out[s, cb * P : (cb + 1) * P, n_off : n_off + nch],
                        ot[:, cb * 512 : cb * 512 + nch],
                    )
            if not last:
                dst = out[s].rearrange("(cb p) n -> p cb n", p=P)[
                    :, :, n_off : n_off + nch
                ]
                src = ot.rearrange("p (cb n) -> p cb n", cb=CB)[:, :, 0:nch]
                nc.sync.dma_start(dst, src)
            n_off += nch
        del v_t[i], vt_t[i]

    load_v_alloc(0)
    load_v_range(0, 0)
    # gamma after the first v range: it is not needed until the epilogue
    nc.sync.dma_start(gamma_sb[:], bass.AP(gamma.tensor, 0, [[0, P], [1, 1]]))
    for r in range(1, len(VB0) - 1):
        load_v_range(0, r)
    for i in range(nsamp):
        emit(i)


_nc_cache = {}


def _build(reps=1):
    if reps in _nc_cache:
        return _nc_cache[reps]
    nc = bacc.Bacc("TRN2", target_bir_lowering=False, debug=False)
    x_d = nc.dram_tensor("x", [SPC, C, N], FP32, kind="ExternalInput")
    g_d = nc.dram_tensor("gamma", [1], FP32, kind="ExternalInput")
    o_d = nc.dram_tensor("out", [SPC, C, N], FP32, kind="ExternalOutput")
    with tile.TileContext(nc) as tc, ExitStack() as ctx:
        _emit(tc, ctx, x_d.ap(), g_d.ap(), o_d.ap(), reps=reps)
    nc.compile()
    _nc_cache[reps] = nc
    return nc


def _bench_fn(reps, x, gamma):
    """Build a jitted 8-core executor for the reps-times-repeated kernel with
    device-resident inputs.  Used by test.py for differential timing."""
    import jax
    from jax.experimental.shard_map import shard_map
    from jax.sharding import Mesh, NamedSharding, PartitionSpec

    from concourse import bass2jax

    bass2jax.install_neuronx_cc_hook()
    nc = _build(reps=reps)
    pid = nc.partition_id_tensor.name if nc.partition_id_tensor else None
    in_names, out_names, out_avals, zero_outs = [], [], [], []
    for alloc in nc.m.functions[0].allocations:
        if not isinstance(alloc, mybir.MemoryLocationSet):
            continue
        name = alloc.memorylocations[0].name
        if alloc.kind == "ExternalInput":
            if name != pid:
                in_names.append(name)
        elif alloc.kind == "ExternalOutput":
            out_names.append(name)
            shape = tuple(alloc.tensor_shape)
            dtype = mybir.dt.np(alloc.dtype)
            out_avals.append(jax.core.ShapedArray(shape, dtype))
            zero_outs.append(np.zeros(shape, dtype))
    all_in_names = list(in_names) + list(out_names)
    if pid:
        all_in_names.append(pid)

    def _body(*args):
        operands = list(args)
        if pid:
            operands.append(bass2jax.partition_id_tensor())
        return tuple(
            bass2jax._bass_exec_p.bind(
                *operands,
                out_avals=tuple(out_avals),
                in_names=tuple(all_in_names),
                out_names=tuple(out_names),
                lowering_input_output_aliases=(),
                sim_require_finite=True,
                sim_require_nnan=True,
                nc=nc,
            )
        )

    devices = jax.devices()[:NCORES]
    mesh = Mesh(np.asarray(devices), ("core",))
    specs = (PartitionSpec("core"),) * (len(in_names) + len(out_names))
    fn = jax.jit(
        shard_map(
            _body,
            mesh=mesh,
            in_specs=specs,
            out_specs=(PartitionSpec("core"),) * len(out_names),
            check_rep=False,
        ),
        keep_unused=True,
    )
    sh = NamedSharding(mesh, PartitionSpec("core"))
    ins = {
        "x": np.ascontiguousarray(x, dtype=np.float32).reshape(B, C, N),
        "gamma": np.tile(np.ascontiguousarray(gamma, dtype=np.float32), (NCORES,)),
    }
    args = [jax.device_put(ins[n], sh) for n in in_names]
    args += [
        jax.device_put(np.zeros((NCORES * z.shape[0], *z.shape[1:]), z.dtype), sh)
        for z in zero_outs
    ]
    return fn, args


def kernel(x: np.ndarray, gamma: np.ndarray, **run_kwargs) -> np.ndarray:
    assert x.shape == (B, C, H, W), x.shape
    nc = _build()
    xr = np.ascontiguousarray(x, dtype=np.float32).reshape(B, C, N)
    g = np.ascontiguousarray(gamma, dtype=np.float32)
    in_maps = [
        {"x": xr[g_idx * SPC : (g_idx + 1) * SPC], "gamma": g}
        for g_idx in range(NCORES)
    ]
    res = run_bass_kernel_spmd(nc, in_maps, core_ids=list(range(NCORES)), **run_kwargs)
    outs = [res.results[g_idx]["out"] for g_idx in range(NCORES)]
    full = np.concatenate(outs, axis=0).reshape(B, C, H, W).astype(np.float32)
    if run_kwargs:
        kernel.last_results = res
    return full


# revision 44
# speedup vs baseline: 1.3718x; 1.3718x over previous
"""CAM (channel attention) module kernel for Trainium2, data-parallel over batch.

Computes, per sample:
    v = x.reshape(C, N)                  # N = H*W
    energy = v @ v.T                     # [C, C]
    att = softmax(rowmax(energy) - energy, axis=-1)
    out = gamma * (att @ v) + x

Distribution: batch B=32 split over 8 NeuronCores (4 samples/core), gamma
replicated.  Per core everything is computed on-chip:
  - v loaded once to SBUF (doubles as x for the residual add)
  - v^T built with PE transpose-mode matmuls (needed for the energy matmul,
    whose contraction is over the spatial axis)
  - energy is SYMMETRIC (E = V V^T), so only the upper-triangle block rows
    are computed (free widths 512/384/256/256 per 128-row block), with
    float32r (FP22) matmuls accumulated in PSUM
  - softmax via a GLOBAL-bias exponent: with any per-sample constant G,
    Q = exp(G - E) is symmetric, so Q doubles as the *transposed*
    unnormalized attention (the per-row max/min bias of the reference
    softmax cancels exactly against the row normalizer).  Row sums of Q
    (ACT accum_out) give Z'.  Then out = (Q @ v) * (gamma / Z') + x.
    This removes the attention transpose + per-row min reductions of the
    direct formulation entirely.
  - G is derived on-chip from block-0's row minima: G = (min+max)/2 - 10,
    computed with one DVE row-reduce + two gpsimd partition_all_reduce ops;
    sample i uses sample i-1's G (the safety window is far wider than the
    cross-sample drift) so the exps never wait on the reduction chain
    (numerics: exponents stay within +-83, Q in [1e-36, 8e35], validated)
  - the 5 lower-triangle Q blocks are mirrored from the upper ones with PE
    transposes; the DVE copy back to SBUF also row-sums them (accum_out) to
    complete Z'
  - epilogue fuses (psum * (gamma/Z')) + x in one DVE pass
"""

import sys

sys.path.insert(0, "/opt/trn_rl_repo")

from contextlib import ExitStack

import numpy as np

import concourse.bacc as bacc
import concourse.bass as bass
import concourse.bass_isa as bass_isa
import concourse.mybir as mybir
import concourse.tile as tile
from concourse import masks
from concourse.bass_utils import run_bass_kernel_spmd

B, C, H, W = 32, 512, 48, 48
N = H * W  # 2304
NCORES = 8
SPC = B // NCORES  # samples per core
P = 128
CB = C // P  # 4 channel blocks
KB = N // P  # 18 spatial chunks of 128
NCH = [512, 512, 512, 512, 256]  # free-dim chunking of N for the 2nd matmul
G_SHIFT = 10.0  # centers exp(G - E) in fp32 range (see module docstring)

# stored (upper-triangle) energy row-block geometry: row ib holds cols
# [CLO[ib]*P, 512) of the attention matrix
CLO = [0, 1, 2, 2]  # first stored 128-col block per row (row 3 keeps 2 blocks
#                     so the matmul free dim stays >= 256, the fp32r fast path)
MIRROR = [(1, 0), (2, 0), (2, 1), (3, 0), (3, 1)]  # (ib, jb): fill from (jb, ib)^T

FP32 = mybir.dt.float32
FP32R = mybir.dt.float32r
AX = mybir.AxisListType.X
OP = mybir.AluOpType
AF = mybir.ActivationFunctionType
RED = bass_isa.ReduceOp


def _emit(tc, ctx, x, gamma, out, reps=1):
    nc = tc.nc

    const_pool = ctx.enter_context(tc.tile_pool(name="const", bufs=1))
    ident_f32 = const_pool.tile([P, P], FP32)
    masks.make_identity(nc, ident_f32[:])
    ident = const_pool.tile([P, P], FP32R)
    nc.scalar.copy(ident[:], ident_f32[:])
    gamma_sb = const_pool.tile([P, 1], FP32)

    v_pool = ctx.enter_context(tc.tile_pool(name="v", bufs=3))
    vt_pool = ctx.enter_context(tc.tile_pool(name="vt", bufs=1))
    q_pool = ctx.enter_context(tc.tile_pool(name="q", bufs=2))
    o_pool = ctx.enter_context(tc.tile_pool(name="o", bufs=3))
    vec_pool = ctx.enter_context(tc.tile_pool(name="vec", bufs=4))
    s_pool = ctx.enter_context(tc.tile_pool(name="s", bufs=2))
    # PSUM budget exactly 8 banks: 4 energy (one per row block), 2 rotating
    # transpose banks (v^T chunks and Q mirrors), 2 output banks.
    ps_e = ctx.enter_context(tc.tile_pool(name="ps_e", bufs=1, space="PSUM"))
    ps_t = ctx.enter_context(tc.tile_pool(name="ps_t", bufs=2, space="PSUM"))
    ps_o = ctx.enter_context(tc.tile_pool(name="ps_o", bufs=2, space="PSUM"))

    nsamp = reps * SPC
    v_t = {}
    vt_t = {}
    g_t = {}

    # v loads are range-major (all 4 c-blocks of a column range together) so
    # the k-th transpose chunk only needs the first few ranges.  The first
    # range is a single 128-col chunk so the first transpose starts early.
    VB = [0, 128] + [128 + 384 * r for r in range(1, 6)] + [N]
    # sample 0's transposes consume chunks just-in-time at ~0.9us each, so
    # its ranges are finer up front to match delivery to consumption
    VB0 = [0, 128, 256, 384, 640, 896, 1280, 1664, 2048, N]

    def load_v_alloc(i):
        v_t[i] = v_pool.tile([P, CB * N], FP32R, tag="v", name=f"v{i}")

    def load_v_range(i, r):
        # ONE dma per range covering all 4 c-blocks (3D access pattern):
        # each dma_start holds the issuing queue's sequencer ~0.6us, so
        # instruction count — not bytes — set the issue rate.  All on the SP
        # queue: ACT must stay DMA-free so the latency-critical exps are
        # never stuck behind a DMA's descriptor-generation hold.
        s, v = i % SPC, v_t[i]
        vb = VB0 if i == 0 else VB
        a, b = vb[r], vb[r + 1]
        dst = v.rearrange("p (cb n) -> p cb n", cb=CB)[:, :, a:b]
        src = x[s].rearrange("(cb p) n -> p cb n", p=P)[:, :, a:b]
        nc.sync.dma_start(dst, src.bitcast(FP32R))

    def a_chunk(i, k, copy_eng=None):
        # transpose one 128-wide spatial chunk of v into vt
        if k == 0:
            vt_t[i] = vt_pool.tile([P, KB * C], FP32R, tag="vt", name=f"vt{i}")
        v, vt = v_t[i], vt_t[i]
        tps = ps_t.tile([P, 512], FP32R, tag="tps")
        for cb in range(CB):
            nc.tensor.matmul(
                tps[:, cb * P : (cb + 1) * P],
                v[:, cb * N + k * P : cb * N + (k + 1) * P],
                ident[:],
                is_transpose=True,
                start=(cb == 0),
                stop=(cb == CB - 1),
            )
        if copy_eng == "dve":
            nc.vector.tensor_copy(vt[:, k * C : (k + 1) * C], tps[:])
        elif copy_eng == "act":
            nc.scalar.copy(vt[:, k * C : (k + 1) * C], tps[:])
        else:
            nc.any.tensor_copy(vt[:, k * C : (k + 1) * C], tps[:])

    def emit(i):
        s = i % SPC
        v, vt = v_t[i], vt_t.get(i)
        if i + 1 < nsamp:
            load_v_alloc(i + 1)

        # ---- upper-triangle energy: row block ib holds cols [CLO[ib]*P, 512)
        # one PSUM bank per row block so starting row ib+1's accumulation
        # group never serializes against row ib's softmax readers
        energy = [
            ps_e.tile([P, 512], FP32, tag=f"eb{ib}", name=f"eb{ib}_{i}")
            for ib in range(CB)
        ]
        q_sb = q_pool.tile([P, CB * 512], FP32R, tag="q")
        z = vec_pool.tile([P, CB], FP32, tag="z")
        s_all = s_pool.tile([P, CB], FP32, tag="s")

        def row_rgn(ib):
            return slice(ib * 512 + CLO[ib] * P, (ib + 1) * 512)

        def energy_mm(ib, k):
            lo = CLO[ib] * P
            nc.tensor.matmul(
                energy[ib][:, lo:512],
                vt_t[i][:, k * C + ib * P : k * C + (ib + 1) * P],
                vt_t[i][:, k * C + lo : k * C + C],
                start=(k == 0),
                stop=(k == KB - 1),
            )

        if i == 0:
            # prologue: v DMAs are still landing — build vt chunks just in
            # time and advance all 4 row blocks per chunk (k-outer) so the
            # PE keeps pace with the DMA stream
            for k in range(KB):
                if k == 0:
                    a_chunk(0, 0)
                if k + 1 < KB:
                    a_chunk(0, k + 1)
                for ib in range(CB):
                    energy_mm(ib, k)
        for ib in range(CB):
            lo = CLO[ib] * P
            if i > 0:
                for k in range(KB):
                    energy_mm(ib, k)
            if ib == 0:
                # ---- global bias G = (min+max of block-0 row minima)/2 - 10
                mn0 = vec_pool.tile([P, 1], FP32, tag="mn0")
                nc.vector.tensor_reduce(
                    mn0[:], energy[0][:, 0:512], axis=AX, op=OP.min
                )
                nm0 = vec_pool.tile([P, 1], FP32, tag="nm0")
                nc.vector.tensor_scalar(nm0[:], mn0[:], -1.0, None, OP.mult)
                mx_r = vec_pool.tile([P, 1], FP32, tag="mx_r")
                nc.gpsimd.partition_all_reduce(mx_r[:], mn0[:], P, RED.max)
                nmn_r = vec_pool.tile([P, 1], FP32, tag="nmn_r")
                nc.gpsimd.partition_all_reduce(nmn_r[:], nm0[:], P, RED.max)
                g_sb = vec_pool.tile([P, 1], FP32, tag="g_sb")
                nc.vector.tensor_tensor(g_sb[:], mx_r[:], nmn_r[:], op=OP.subtract)
                nc.vector.tensor_scalar(
                    g_sb[:], g_sb[:], 0.5, -G_SHIFT, OP.mult, OP.add
                )
                g_t[i] = g_sb
            # ---- Q = exp(G - E) over the stored region; accum -> partial Z'
            # G comes from the PREVIOUS sample (validated numerically: the
            # safety window is far wider than the cross-sample drift), so the
            # exps never wait on this sample's G reduction chain
            if i == 0 and ib == 0:
                # sample 0 has no lagged G: everything downstream waits on
                # this exp, and the first mirror/output matmuls only need its
                # left half — split it so they start half an exp earlier
                zh = vec_pool.tile([P, 2], FP32, tag="zh")
                for h in (0, 1):
                    nc.scalar.activation(
                        q_sb[:, ib * 512 + h * 256 : ib * 512 + (h + 1) * 256],
                        energy[ib][:, h * 256 : (h + 1) * 256],
                        AF.Exp,
                        bias=g_t[0][:],
                        scale=-1.0,
                        accum_out=zh[:, h : h + 1],
                    )
                nc.vector.tensor_tensor(
                    z[:, 0:1], zh[:, 0:1], zh[:, 1:2], op=OP.add
                )
            else:
                nc.scalar.activation(
                    q_sb[:, row_rgn(ib)],
                    energy[ib][:, lo:512],
                    AF.Exp,
                    bias=g_t[max(i - 1, 0)][:],
                    scale=-1.0,
                    accum_out=z[:, ib : ib + 1],
                )

        # a few of the next sample's transposes fill the exp tail; their
        # PSUM->SBUF copies go on ACT (after the exps in its stream) so the
        # DVE can run the mirror copies that gate the first output block
        if i + 1 < nsamp:
            for r in range(3):
                load_v_range(i + 1, r)
            a_queue = list(range(KB))
            for _ in range(3):
                a_chunk(i + 1, a_queue.pop(0), copy_eng="act")
        else:
            a_queue = []

        # ---- mirror lower-triangle Q blocks (Q is symmetric); the copy back
        # also row-sums them to complete Z'
        pps = {}
        for ib, jb in MIRROR:
            tps = ps_t.tile([P, 512], FP32R, tag="tps")
            nc.tensor.matmul(
                tps[:, 0:P],
                q_sb[:, jb * 512 + ib * P : jb * 512 + (ib + 1) * P],
                ident[:],
                is_transpose=True,
                start=True,
                stop=True,
            )
            pp = vec_pool.tile([P, 1], FP32, tag=f"pp{ib}{jb}")
            nc.vector.tensor_scalar(
                q_sb[:, ib * 512 + jb * P : ib * 512 + (jb + 1) * P],
                tps[:, 0:P],
                0.0,
                None,
                OP.add,
                OP.add,  # accum reduction op
                accum_out=pp[:],
            )
            pps.setdefault(ib, []).append(pp)

        # ---- s = gamma / Z'
        for ib in range(CB):
            zt = z[:, ib : ib + 1]
            for pp in pps.get(ib, []):
                zn = vec_pool.tile([P, 1], FP32, tag=f"zn{ib}")
                nc.vector.tensor_tensor(zn[:], zt, pp[:], op=OP.add)
                zt = zn[:]
            r = vec_pool.tile([P, 1], FP32, tag=f"r{ib}")
            nc.vector.reciprocal(r[:], zt)
            nc.vector.tensor_tensor(
                s_all[:, ib : ib + 1], r[:], gamma_sb[:], op=OP.mult
            )

        # ---- out = (Q^T-as-stored @ v) * (gamma/Z') + x
        # chunk-outer so the 4 c-blocks of one n-chunk can be stored with a
        # SINGLE combined DMA (3D access pattern): 5 output DMAs per sample
        # instead of 20 keeps the SP issue queue (~0.6us/dma) off the
        # critical path
        last = i == nsamp - 1
        n_off = 0
        for ich, nch in enumerate(NCH):
            ot = o_pool.tile([P, CB * 512], FP32, tag="ot")
            for cb in range(CB):
                it = ich * CB + cb
                if i + 1 < nsamp and it in (2, 5, 8, 11):
                    load_v_range(i + 1, 3 + (it - 2) // 3)
                if a_queue:
                    # alternate the PSUM->SBUF copies between ACT and DVE so
                    # neither queue's backlog delays the next sample's energy
                    a_chunk(i + 1, a_queue.pop(0), copy_eng="act" if it % 2 else "dve")
                if last:
                    # no next-sample fillers exist to pace the PE, so rotate
                    # the output accumulators through the freed energy banks
                    # (4-deep) instead of the 2 ps_o banks
                    po = ps_e.tile([P, 512], FP32, tag=f"eb{it % CB}")
                else:
                    po = ps_o.tile([P, 512], FP32, tag="po")
                for db in range(CB):
                    nc.tensor.matmul(
                        po[:, :nch],
                        q_sb[:, db * 512 + cb * P : db * 512 + (cb + 1) * P],
                        v[:, db * N + n_off : db * N + n_off + nch],
                        start=(db == 0),
                        stop=(db == CB - 1),
                    )
                nc.vector.scalar_tensor_tensor(
                    ot[:, cb * 512 : cb * 512 + nch],
                    po[:, :nch],
                    s_all[:, cb : cb + 1],
                    v[:, cb * N + n_off : cb * N + n_off + nch].bitcast(FP32),
                    op0=OP.mult,
                    op1=OP.add,
                )
                if last:
                    # per-block stores at the end: the kernel tail is the last
                    # epilogue->DMA chain, so don't serialize 4 epilogues
                    # into one big store (SP has issue slack by now)
                    nc.sync.dma_start(
                        out[s, cb * P : (cb + 1) * P, n_off : n_off + nch],
                        ot[:, cb * 512 : cb * 512 + nch],
                    )
            if not last:
                dst = out[s].rearrange("(cb p) n -> p cb n", p=P)[
                    :, :, n_off : n_off + nch
                ]
                src = ot.rearrange("p (cb n) -> p cb n", cb=CB)[:, :, 0:nch]
                nc.sync.dma_start(dst, src)
            n_off += nch
        del v_t[i], vt_t[i]

    load_v_alloc(0)
    load_v_range(0, 0)
    # gamma after the first v range: it is not needed until the epilogue
    nc.sync.dma_start(gamma_sb[:], bass.AP(gamma.tensor, 0, [[0, P], [1, 1]]))
    for r in range(1, len(VB0) - 1):
        load_v_range(0, r)
    for i in range(nsamp):
        emit(i)


_nc_cache = {}


def _build(reps=1):
    if reps in _nc_cache:
        return _nc_cache[reps]
    nc = bacc.Bacc("TRN2", target_bir_lowering=False, debug=False)
    x_d = nc.dram_tensor("x", [SPC, C, N], FP32, kind="ExternalInput")
    g_d = nc.dram_tensor("gamma", [1], FP32, kind="ExternalInput")
    o_d = nc.dram_tensor("out", [SPC, C, N], FP32, kind="ExternalOutput")
    with tile.TileContext(nc) as tc, ExitStack() as ctx:
        _emit(tc, ctx, x_d.ap(), g_d.ap(), o_d.ap(), reps=reps)
    nc.compile()
    _nc_cache[reps] = nc
    return nc


def _bench_fn(reps, x, gamma):
    """Build a jitted 8-core executor for the reps-times-repeated kernel with
    device-resident inputs.  Used by test.py for differential timing."""
    import jax
    from jax.experimental.shard_map import shard_map
    from jax.sharding import Mesh, NamedSharding, PartitionSpec

    from concourse import bass2jax

    bass2jax.install_neuronx_cc_hook()
    nc = _build(reps=reps)
    pid = nc.partition_id_tensor.name if nc.partition_id_tensor else None
    in_names, out_names, out_avals, zero_outs = [], [], [], []
    for alloc in nc.m.functions[0].allocations:
        if not isinstance(alloc, mybir.MemoryLocationSet):
            continue
        name = alloc.memorylocations[0].name
        if alloc.kind == "ExternalInput":
            if name != pid:
                in_names.append(name)
        elif alloc.kind == "ExternalOutput":
            out_names.append(name)
            shape = tuple(alloc.tensor_shape)
            dtype = mybir.dt.np(alloc.dtype)
            out_avals.append(jax.core.ShapedArray(shape, dtype))
            zero_outs.append(np.zeros(shape, dtype))
    all_in_names = list(in_names) + list(out_names)
    if pid:
        all_in_names.append(pid)

    def _body(*args):
        operands = list(args)
        if pid:
            operands.append(bass2jax.partition_id_tensor())
        return tuple(
            bass2jax._bass_exec_p.bind(
                *operands,
                out_avals=tuple(out_avals),
                in_names=tuple(all_in_names),
                out_names=tuple(out_names),
                lowering_input_output_aliases=(),
                sim_require_finite=True,
                sim_require_nnan=True,
                nc=nc,
            )
        )

    devices = jax.devices()[:NCORES]
    mesh = Mesh(np.asarray(devices), ("core",))
    specs = (PartitionSpec("core"),) * (len(in_names) + len(out_names))
    fn = jax.jit(
        shard_map(
            _body,
            mesh=mesh,
            in_specs=specs,
            out_specs=(PartitionSpec("core"),) * len(out_names),
            check_rep=False,
        ),
        keep_unused=True,
    )
    sh = NamedSharding(mesh, PartitionSpec("core"))
    ins = {
        "x": np.ascontiguousarray(x, dtype=np.float32).reshape(B, C, N),
        "gamma": np.tile(np.ascontiguousarray(gamma, dtype=np.float32), (NCORES,)),
    }
    args = [jax.device_put(ins[n], sh) for n in in_names]
    args += [
        jax.device_put(np.zeros((NCORES * z.shape[0], *z.shape[1:]), z.dtype), sh)
        for z in zero_outs
    ]
    return fn, args


def kernel(x: np.ndarray, gamma: np.ndarray, **run_kwargs) -> np.ndarray:
    assert x.shape == (B, C, H, W), x.shape
    nc = _build()
    xr = np.ascontiguousarray(x, dtype=np.float32).reshape(B, C, N)
    g = np.ascontiguousarray(gamma, dtype=np.float32)
    in_maps = [
        {"x": xr[g_idx * SPC : (g_idx + 1) * SPC], "gamma": g}
        for g_idx in range(NCORES)
    ]
    res = run_bass_kernel_spmd(nc, in_maps, core_ids=list(range(NCORES)), **run_kwargs)
    outs = [res.results[g_idx]["out"] for g_idx in range(NCORES)]
    full = np.concatenate(outs, axis=0).reshape(B, C, H, W).astype(np.float32)
    if run_kwargs:
        kernel.last_results = res
    return full


# revision 50
# speedup vs baseline: 1.3973x; 1.0186x over previous
"""CAM (channel attention) module kernel for Trainium2, data-parallel over batch.

Computes, per sample:
    v = x.reshape(C, N)                  # N = H*W
    energy = v @ v.T                     # [C, C]
    att = softmax(rowmax(energy) - energy, axis=-1)
    out = gamma * (att @ v) + x

Distribution: batch B=32 split over 8 NeuronCores (4 samples/core), gamma
replicated.  Per core everything is computed on-chip:
  - v loaded once to SBUF (doubles as x for the residual add)
  - v^T built with PE transpose-mode matmuls (needed for the energy matmul,
    whose contraction is over the spatial axis)
  - energy is SYMMETRIC (E = V V^T), so only the upper-triangle block rows
    are computed (free widths 512/384/256/256 per 128-row block), with
    float32r (FP22) matmuls accumulated in PSUM
  - softmax via a GLOBAL-bias exponent: with any per-sample constant G,
    Q = exp(G - E) is symmetric, so Q doubles as the *transposed*
    unnormalized attention (the per-row max/min bias of the reference
    softmax cancels exactly against the row normalizer).  Row sums of Q
    (ACT accum_out) give Z'.  Then out = (Q @ v) * (gamma / Z') + x.
    This removes the attention transpose + per-row min reductions of the
    direct formulation entirely.
  - G is derived on-chip from block-0's row minima: G = (min+max)/2 - 10,
    computed with one DVE row-reduce + two gpsimd partition_all_reduce ops;
    sample i uses sample i-1's G (the safety window is far wider than the
    cross-sample drift) so the exps never wait on the reduction chain
    (numerics: exponents stay within +-83, Q in [1e-36, 8e35], validated)
  - the 5 lower-triangle Q blocks are mirrored from the upper ones with PE
    transposes; the DVE copy back to SBUF also row-sums them (accum_out) to
    complete Z'
  - epilogue fuses (psum * (gamma/Z')) + x in one DVE pass
"""

import sys

sys.path.insert(0, "/opt/trn_rl_repo")

from contextlib import ExitStack

import numpy as np

import concourse.bacc as bacc
import concourse.bass as bass
import concourse.bass_isa as bass_isa
import concourse.mybir as mybir
import concourse.tile as tile
from concourse import masks
from concourse.bass_utils import run_bass_kernel_spmd

B, C, H, W = 32, 512, 48, 48
N = H * W  # 2304
NCORES = 8
SPC = B // NCORES  # samples per core
P = 128
CB = C // P  # 4 channel blocks
KB = N // P  # 18 spatial chunks of 128
NCH = [512, 512, 512, 512, 256]  # free-dim chunking of N for the 2nd matmul
G_SHIFT = 10.0  # centers exp(G - E) in fp32 range (see module docstring)

# stored (upper-triangle) energy row-block geometry: row ib holds cols
# [CLO[ib]*P, 512) of the attention matrix
CLO = [0, 1, 2, 2]  # first stored 128-col block per row (row 3 keeps 2 blocks
#                     so the matmul free dim stays >= 256, the fp32r fast path)
MIRROR = [(1, 0), (2, 0), (2, 1), (3, 0), (3, 1)]  # (ib, jb): fill from (jb, ib)^T

FP32 = mybir.dt.float32
FP32R = mybir.dt.float32r
AX = mybir.AxisListType.X
OP = mybir.AluOpType
AF = mybir.ActivationFunctionType
RED = bass_isa.ReduceOp


def _emit(tc, ctx, x, gamma, out, reps=1):
    nc = tc.nc

    const_pool = ctx.enter_context(tc.tile_pool(name="const", bufs=1))
    ident_f32 = const_pool.tile([P, P], FP32)
    masks.make_identity(nc, ident_f32[:])
    ident = const_pool.tile([P, P], FP32R)
    nc.scalar.copy(ident[:], ident_f32[:])
    gamma_sb = const_pool.tile([P, 1], FP32)

    v_pool = ctx.enter_context(tc.tile_pool(name="v", bufs=3))
    vt_pool = ctx.enter_context(tc.tile_pool(name="vt", bufs=1))
    q_pool = ctx.enter_context(tc.tile_pool(name="q", bufs=2))
    o_pool = ctx.enter_context(tc.tile_pool(name="o", bufs=3))
    vec_pool = ctx.enter_context(tc.tile_pool(name="vec", bufs=4))
    s_pool = ctx.enter_context(tc.tile_pool(name="s", bufs=2))
    # PSUM budget exactly 8 banks: 4 energy (one per row block), 2 rotating
    # transpose banks (v^T chunks and Q mirrors), 2 output banks.
    ps_e = ctx.enter_context(tc.tile_pool(name="ps_e", bufs=1, space="PSUM"))
    ps_t = ctx.enter_context(tc.tile_pool(name="ps_t", bufs=2, space="PSUM"))
    ps_o = ctx.enter_context(tc.tile_pool(name="ps_o", bufs=2, space="PSUM"))

    nsamp = reps * SPC
    v_t = {}
    vt_t = {}
    g_t = {}

    # v loads are range-major (all 4 c-blocks of a column range together) so
    # the k-th transpose chunk only needs the first few ranges.  The first
    # range is a single 128-col chunk so the first transpose starts early.
    VB = [0, 128] + [128 + 384 * r for r in range(1, 6)] + [N]
    # sample 0's transposes consume chunks just-in-time at ~0.9us each, so
    # its ranges are finer up front to match delivery to consumption
    VB0 = [0, 128, 256, 384, 640, 896, 1280, 1664, 2048, N]

    def load_v_alloc(i):
        v_t[i] = v_pool.tile([P, CB * N], FP32R, tag="v", name=f"v{i}")

    def load_v_range(i, r):
        # ONE dma per range covering all 4 c-blocks (3D access pattern):
        # each dma_start holds the issuing queue's sequencer ~0.6us, so
        # instruction count — not bytes — set the issue rate.  All on the SP
        # queue: ACT must stay DMA-free so the latency-critical exps are
        # never stuck behind a DMA's descriptor-generation hold.
        s, v = i % SPC, v_t[i]
        vb = VB0 if i == 0 else VB
        a, b = vb[r], vb[r + 1]
        dst = v.rearrange("p (cb n) -> p cb n", cb=CB)[:, :, a:b]
        src = x[s].rearrange("(cb p) n -> p cb n", p=P)[:, :, a:b]
        nc.sync.dma_start(dst, src.bitcast(FP32R))

    def a_chunk(i, k, copy_eng=None):
        # transpose one 128-wide spatial chunk of v into vt
        if k == 0:
            vt_t[i] = vt_pool.tile([P, KB * C], FP32R, tag="vt", name=f"vt{i}")
        v, vt = v_t[i], vt_t[i]
        tps = ps_t.tile([P, 512], FP32R, tag="tps")
        for cb in range(CB):
            nc.tensor.matmul(
                tps[:, cb * P : (cb + 1) * P],
                v[:, cb * N + k * P : cb * N + (k + 1) * P],
                ident[:],
                is_transpose=True,
                start=(cb == 0),
                stop=(cb == CB - 1),
            )
        if copy_eng == "dve":
            nc.vector.tensor_copy(vt[:, k * C : (k + 1) * C], tps[:])
        elif copy_eng == "act":
            nc.scalar.copy(vt[:, k * C : (k + 1) * C], tps[:])
        else:
            nc.any.tensor_copy(vt[:, k * C : (k + 1) * C], tps[:])

    def emit(i):
        s = i % SPC
        v, vt = v_t[i], vt_t.get(i)
        if i + 1 < nsamp:
            load_v_alloc(i + 1)

        # ---- upper-triangle energy: row block ib holds cols [CLO[ib]*P, 512)
        # one PSUM bank per row block so starting row ib+1's accumulation
        # group never serializes against row ib's softmax readers
        energy = [
            ps_e.tile([P, 512], FP32, tag=f"eb{ib}", name=f"eb{ib}_{i}")
            for ib in range(CB)
        ]
        q_sb = q_pool.tile([P, CB * 512], FP32R, tag="q")
        z = vec_pool.tile([P, CB], FP32, tag="z")
        s_all = s_pool.tile([P, CB], FP32, tag="s")

        def row_rgn(ib):
            return slice(ib * 512 + CLO[ib] * P, (ib + 1) * 512)

        def energy_mm(ib, k):
            lo = CLO[ib] * P
            nc.tensor.matmul(
                energy[ib][:, lo:512],
                vt_t[i][:, k * C + ib * P : k * C + (ib + 1) * P],
                vt_t[i][:, k * C + lo : k * C + C],
                start=(k == 0),
                stop=(k == KB - 1),
            )

        if i == 0:
            # prologue: v DMAs are still landing — build vt chunks just in
            # time and advance all 4 row blocks per chunk (k-outer) so the
            # PE keeps pace with the DMA stream
            for k in range(KB):
                if k == 0:
                    a_chunk(0, 0)
                if k + 1 < KB:
                    a_chunk(0, k + 1)
                for ib in range(CB):
                    energy_mm(ib, k)
        for ib in range(CB):
            lo = CLO[ib] * P
            if i > 0:
                for k in range(KB):
                    energy_mm(ib, k)
            if ib == 0:
                # ---- global bias G = (min+max of block-0 row minima)/2 - 10
                mn0 = vec_pool.tile([P, 1], FP32, tag="mn0")
                nc.vector.tensor_reduce(
                    mn0[:], energy[0][:, 0:512], axis=AX, op=OP.min
                )
                nm0 = vec_pool.tile([P, 1], FP32, tag="nm0")
                nc.vector.tensor_scalar(nm0[:], mn0[:], -1.0, None, OP.mult)
                mx_r = vec_pool.tile([P, 1], FP32, tag="mx_r")
                nc.gpsimd.partition_all_reduce(mx_r[:], mn0[:], P, RED.max)
                nmn_r = vec_pool.tile([P, 1], FP32, tag="nmn_r")
                nc.gpsimd.partition_all_reduce(nmn_r[:], nm0[:], P, RED.max)
                g_sb = vec_pool.tile([P, 1], FP32, tag="g_sb")
                nc.vector.tensor_tensor(g_sb[:], mx_r[:], nmn_r[:], op=OP.subtract)
                nc.vector.tensor_scalar(
                    g_sb[:], g_sb[:], 0.5, -G_SHIFT, OP.mult, OP.add
                )
                g_t[i] = g_sb
            # ---- Q = exp(G - E) over the stored region; accum -> partial Z'
            # G comes from the PREVIOUS sample (validated numerically: the
            # safety window is far wider than the cross-sample drift), so the
            # exps never wait on this sample's G reduction chain
            if i == 0 and ib == 0:
                # sample 0 has no lagged G: everything downstream waits on
                # this exp, and the first mirror/output matmuls only need its
                # left half — split it so they start half an exp earlier
                zh = vec_pool.tile([P, 2], FP32, tag="zh")
                for h in (0, 1):
                    nc.scalar.activation(
                        q_sb[:, ib * 512 + h * 256 : ib * 512 + (h + 1) * 256],
                        energy[ib][:, h * 256 : (h + 1) * 256],
                        AF.Exp,
                        bias=g_t[0][:],
                        scale=-1.0,
                        accum_out=zh[:, h : h + 1],
                    )
                nc.vector.tensor_tensor(
                    z[:, 0:1], zh[:, 0:1], zh[:, 1:2], op=OP.add
                )
            else:
                nc.scalar.activation(
                    q_sb[:, row_rgn(ib)],
                    energy[ib][:, lo:512],
                    AF.Exp,
                    bias=g_t[max(i - 1, 0)][:],
                    scale=-1.0,
                    accum_out=z[:, ib : ib + 1],
                )

        # a few of the next sample's transposes fill the exp tail; their
        # PSUM->SBUF copies go on ACT (after the exps in its stream) so the
        # DVE can run the mirror copies that gate the first output block
        if i + 1 < nsamp:
            for r in range(3):
                load_v_range(i + 1, r)
            a_queue = list(range(KB))
            for _ in range(3):
                a_chunk(i + 1, a_queue.pop(0), copy_eng="act")
        else:
            a_queue = []

        # ---- mirror lower-triangle Q blocks (Q is symmetric); the copy back
        # also row-sums them to complete Z'
        pps = {}
        for ib, jb in MIRROR:
            tps = ps_t.tile([P, 512], FP32R, tag="tps")
            nc.tensor.matmul(
                tps[:, 0:P],
                q_sb[:, jb * 512 + ib * P : jb * 512 + (ib + 1) * P],
                ident[:],
                is_transpose=True,
                start=True,
                stop=True,
            )
            pp = vec_pool.tile([P, 1], FP32, tag=f"pp{ib}{jb}")
            nc.vector.tensor_scalar(
                q_sb[:, ib * 512 + jb * P : ib * 512 + (jb + 1) * P],
                tps[:, 0:P],
                0.0,
                None,
                OP.add,
                OP.add,  # accum reduction op
                accum_out=pp[:],
            )
            pps.setdefault(ib, []).append(pp)

        # ---- s = gamma / Z'
        for ib in range(CB):
            zt = z[:, ib : ib + 1]
            for pp in pps.get(ib, []):
                zn = vec_pool.tile([P, 1], FP32, tag=f"zn{ib}")
                nc.vector.tensor_tensor(zn[:], zt, pp[:], op=OP.add)
                zt = zn[:]
            r = vec_pool.tile([P, 1], FP32, tag=f"r{ib}")
            nc.vector.reciprocal(r[:], zt)
            nc.vector.tensor_tensor(
                s_all[:, ib : ib + 1], r[:], gamma_sb[:], op=OP.mult
            )

        # ---- out = (Q^T-as-stored @ v) * (gamma/Z') + x
        # chunk-outer so the 4 c-blocks of one n-chunk can be stored with a
        # SINGLE combined DMA (3D access pattern): 5 output DMAs per sample
        # instead of 20 keeps the SP issue queue (~0.6us/dma) off the
        # critical path
        last = i == nsamp - 1
        n_off = 0
        for ich, nch in enumerate(NCH):
            ot = o_pool.tile([P, CB * 512], FP32, tag="ot")
            for cb in range(CB):
                it = ich * CB + cb
                if i + 1 < nsamp and it in (2, 5, 8, 11):
                    load_v_range(i + 1, 3 + (it - 2) // 3)
                if a_queue:
                    # alternate the PSUM->SBUF copies between ACT and DVE so
                    # neither queue's backlog delays the next sample's energy
                    a_chunk(i + 1, a_queue.pop(0), copy_eng="act" if it % 2 else "dve")
                if last:
                    # no next-sample fillers exist to pace the PE, so rotate
                    # the output accumulators through the freed energy banks
                    # (4-deep) instead of the 2 ps_o banks
                    po = ps_e.tile([P, 512], FP32, tag=f"eb{it % CB}")
                else:
                    po = ps_o.tile([P, 512], FP32, tag="po")
                for db in range(CB):
                    nc.tensor.matmul(
                        po[:, :nch],
                        q_sb[:, db * 512 + cb * P : db * 512 + (cb + 1) * P],
                        v[:, db * N + n_off : db * N + n_off + nch],
                        start=(db == 0),
                        stop=(db == CB - 1),
                    )
                nc.vector.scalar_tensor_tensor(
                    ot[:, cb * 512 : cb * 512 + nch],
                    po[:, :nch],
                    s_all[:, cb : cb + 1],
                    v[:, cb * N + n_off : cb * N + n_off + nch].bitcast(FP32),
                    op0=OP.mult,
                    op1=OP.add,
                )
                if last:
                    # per-block stores at the end: the kernel tail is the last
                    # epilogue->DMA chain, so don't serialize 4 epilogues
                    # into one big store (SP has issue slack by now)
                    nc.sync.dma_start(
                        out[s, cb * P : (cb + 1) * P, n_off : n_off + nch],
                        ot[:, cb * 512 : cb * 512 + nch],
                    )
            if not last:
                dst = out[s].rearrange("(cb p) n -> p cb n", p=P)[
                    :, :, n_off : n_off + nch
                ]
                src = ot.rearrange("p (cb n) -> p cb n", cb=CB)[:, :, 0:nch]
                nc.sync.dma_start(dst, src)
            n_off += nch
        del v_t[i], vt_t[i]

    load_v_alloc(0)
    load_v_range(0, 0)
    # gamma after the first v range: it is not needed until the epilogue
    nc.sync.dma_start(gamma_sb[:], bass.AP(gamma.tensor, 0, [[0, P], [1, 1]]))
    for r in range(1, len(VB0) - 1):
        load_v_range(0, r)
    for i in range(nsamp):
        emit(i)


_nc_cache = {}


def _build(reps=1):
    if reps in _nc_cache:
        return _nc_cache[reps]
    nc = bacc.Bacc("TRN2", target_bir_lowering=False, debug=False)
    x_d = nc.dram_tensor("x", [SPC, C, N], FP32, kind="ExternalInput")
    g_d = nc.dram_tensor("gamma", [1], FP32, kind="ExternalInput")
    o_d = nc.dram_tensor("out", [SPC, C, N], FP32, kind="ExternalOutput")
    with tile.TileContext(nc) as tc, ExitStack() as ctx:
        _emit(tc, ctx, x_d.ap(), g_d.ap(), o_d.ap(), reps=reps)
    nc.compile()
    _nc_cache[reps] = nc
    return nc


def _bench_fn(reps, x, gamma):
    """Build a jitted 8-core executor for the reps-times-repeated kernel with
    device-resident inputs.  Used by test.py for differential timing."""
    import jax
    from jax.experimental.shard_map import shard_map
    from jax.sharding import Mesh, NamedSharding, PartitionSpec

    from concourse import bass2jax

    bass2jax.install_neuronx_cc_hook()
    nc = _build(reps=reps)
    pid = nc.partition_id_tensor.name if nc.partition_id_tensor else None
    in_names, out_names, out_avals, zero_outs = [], [], [], []
    for alloc in nc.m.functions[0].allocations:
        if not isinstance(alloc, mybir.MemoryLocationSet):
            continue
        name = alloc.memorylocations[0].name
        if alloc.kind == "ExternalInput":
            if name != pid:
                in_names.append(name)
        elif alloc.kind == "ExternalOutput":
            out_names.append(name)
            shape = tuple(alloc.tensor_shape)
            dtype = mybir.dt.np(alloc.dtype)
            out_avals.append(jax.core.ShapedArray(shape, dtype))
            zero_outs.append(np.zeros(shape, dtype))
    all_in_names = list(in_names) + list(out_names)
    if pid:
        all_in_names.append(pid)

    def _body(*args):
        operands = list(args)
        if pid:
            operands.append(bass2jax.partition_id_tensor())
        return tuple(
            bass2jax._bass_exec_p.bind(
                *operands,
                out_avals=tuple(out_avals),
                in_names=tuple(all_in_names),
                out_names=tuple(out_names),
                lowering_input_output_aliases=(),
                sim_require_finite=True,
                sim_require_nnan=True,
                nc=nc,
            )
        )

    devices = jax.devices()[:NCORES]
    mesh = Mesh(np.asarray(devices), ("core",))
    specs = (PartitionSpec("core"),) * (len(in_names) + len(out_names))
    fn = jax.jit(
        shard_map(
            _body,
            mesh=mesh,
            in_specs=specs,
            out_specs=(PartitionSpec("core"),) * len(out_names),
            check_rep=False,
        ),
        keep_unused=True,
    )
    sh = NamedSharding(mesh, PartitionSpec("core"))
    ins = {
        "x": np.ascontiguousarray(x, dtype=np.float32).reshape(B, C, N),
        "gamma": np.tile(np.ascontiguousarray(gamma, dtype=np.float32), (NCORES,)),
    }
    args = [jax.device_put(ins[n], sh) for n in in_names]
    args += [
        jax.device_put(np.zeros((NCORES * z.shape[0], *z.shape[1:]), z.dtype), sh)
        for z in zero_outs
    ]
    return fn, args


def kernel(x: np.ndarray, gamma: np.ndarray, **run_kwargs) -> np.ndarray:
    assert x.shape == (B, C, H, W), x.shape
    nc = _build()
    xr = np.ascontiguousarray(x, dtype=np.float32).reshape(B, C, N)
    g = np.ascontiguousarray(gamma, dtype=np.float32)
    in_maps = [
        {"x": xr[g_idx * SPC : (g_idx + 1) * SPC], "gamma": g}
        for g_idx in range(NCORES)
    ]
    res = run_bass_kernel_spmd(nc, in_maps, core_ids=list(range(NCORES)), **run_kwargs)
    outs = [res.results[g_idx]["out"] for g_idx in range(NCORES)]
    full = np.concatenate(outs, axis=0).reshape(B, C, H, W).astype(np.float32)
    if run_kwargs:
        kernel.last_results = res
    return full
